# revision 5
# baseline (speedup 1.0000x reference)
"""CRF forward-algorithm loss on 8 Trainium2 NeuronCores — bidirectional.

Math: reference computes logZ of a 2048-state CRF over 8192 steps.
We split the chain in the middle:
  fwd:  w_{t+1} = (aA w_t) o e_t * r        t = 0..4095   (A[n,p]=exp(T[n,p]))
  bwd:  y_{t+1} = (aA^T y_t) o eb_t * r     t = 0..4095   (eb = reversed e, last = 1)
  logZ = log(w_F . y_B) - sum log(applied scales) - 2*4096*log(a)
Both chains run INTERLEAVED on all 8 cores (tensor-parallel over `next`,
256 nexts/core); each chain's serial latency (psum evac, transpose,
broadcast flight) hides under the other chain's matvec.

Per-core, per-round (= 1 fwd + 1 bwd step), raw bass:
  PE : 8 accumulating fp8e5 DoubleRow matmuls [256c x 1] x [256c x 257]
       -> psum row [1,257] per chain; then per chain 2 transpose matmuls
       (stationary = bf16 row half [1,128], rhs = stale 1/S scalar) -> [128,2]
  ACT: psum row -> bf16 rowbuf copy (per chain)
  DVE: psum_t -> send tile (bf16), 1/S recip + record, w' = gather x exp(h)
       (output cast to fp8e5)
  GP : 2 remote_dma_broadcast preps + triggers ([128,2] bf16 to all 8 cores)
  SP : exp(h) block prefetch
Scaling is stale-by-one: step t applies r_{t-1} (slot t of the rec array,
slot 0 preset to 1.0), recorded exactly; host subtracts the logs.
"""

import sys

if "/opt/trn_rl_repo" not in sys.path:
    sys.path.insert(0, "/opt/trn_rl_repo")

import numpy as np
import ml_dtypes

import concourse.bass as bass
import concourse.bacc as bacc
import concourse.mybir as mybir

START_IDX = 0
END_IDX = 1
K = 2048
SEQ = 8192
SEQH = SEQ // 2
NCORES = 8
P = 128
SLICE = K // NCORES      # 256 nexts per core
MT = K // P              # 16 contract chunks of 128
NMM = MT // 2            # 8 DoubleRow matmuls per matvec
MCOLS = SLICE + 1        # 256 nexts + 1 colsum column
MPAD = 272               # padded per-k-tile moving block (16B aligned)
MO = 2 * MPAD            # per-matmul moving stride
ALPHA = float(2.0 ** -11)
BETA = 16.0              # emission pre-scale (conditioning only)
BF16 = mybir.dt.bfloat16
F32 = mybir.dt.float32
FP8 = mybir.dt.float8e5
NPBF16 = ml_dtypes.bfloat16
NPFP8 = ml_dtypes.float8_e5m2


def build_bass(seq_blocks: int, blk_steps: int, variant: str = "full") -> bass.Bass:
    assert blk_steps % 2 == 0 and seq_blocks % 2 == 0
    comm = variant in ("full", "sim", "norswait")
    rswait = variant != "norswait"
    simulate = variant in ("sim", "simnc")  # TimelineSim: no barriers
    if variant == "simnc":
        comm = False
    trans = variant in ("full", "nocomm", "sim", "simnc", "norswait")
    seqh = seq_blocks * blk_steps
    nc = bacc.Bacc(None, target_bir_lowering=False, num_devices=NCORES)

    # ---- params ----
    movf = nc.declare_dram_parameter("movf", [P, NMM * MO], FP8, isOutput=False)
    movb = nc.declare_dram_parameter("movb", [P, NMM * MO], FP8, isOutput=False)
    hqf = nc.declare_dram_parameter("hqf", [P, seqh * MT], BF16, isOutput=False)
    hqb = nc.declare_dram_parameter("hqb", [P, seqh * MT], BF16, isOutput=False)
    wfi = nc.declare_dram_parameter("wfi", [P, 32], FP8, isOutput=False)
    wbi = nc.declare_dram_parameter("wbi", [P, 32], FP8, isOutput=False)
    wfo = nc.declare_dram_parameter("wfo", [P, 32], FP8, isOutput=True)
    wbo = nc.declare_dram_parameter("wbo", [P, 32], FP8, isOutput=True)
    recf_o = nc.declare_dram_parameter("recf", [1, seqh + 2], BF16, isOutput=True)
    recb_o = nc.declare_dram_parameter("recb", [1, seqh + 2], BF16, isOutput=True)

    # ---- sbuf ----
    movf_sb = nc.alloc_sbuf_tensor("movf_sb", [P, NMM * MO], FP8)
    movb_sb = nc.alloc_sbuf_tensor("movb_sb", [P, NMM * MO], FP8)
    wf_sb = nc.alloc_sbuf_tensor("wf_sb", [P, 32], FP8)
    wb_sb = nc.alloc_sbuf_tensor("wb_sb", [P, 32], FP8)
    hqf_sb = nc.alloc_sbuf_tensor("hqf_sb", [P, 2 * blk_steps * MT], BF16)
    hqb_sb = nc.alloc_sbuf_tensor("hqb_sb", [P, 2 * blk_steps * MT], BF16)
    rowf = nc.alloc_sbuf_tensor("rowf", [1, 512], BF16)      # parity halves
    rowb = nc.alloc_sbuf_tensor("rowb", [1, 512], BF16)
    # combined send tile: [par 2][f 2 | b 2]; combined gather: [par 2][slot 8][f 2 | b 2]
    sendc = nc.alloc_sbuf_tensor("sendc", [P, 8], BF16)
    grawc = nc.alloc_sbuf_tensor("grawc", [P, 64], BF16)
    recf_sb = nc.alloc_sbuf_tensor("recf_sb", [1, seqh + 2], BF16)
    recb_sb = nc.alloc_sbuf_tensor("recb_sb", [1, seqh + 2], BF16)


    psf = [nc.alloc_psum_tensor(f"psf{i}", [P, 512], F32) for i in range(2)]
    psb = [nc.alloc_psum_tensor(f"psb{i}", [P, 512], F32) for i in range(2)]
    pstf = nc.alloc_psum_tensor("pstf", [P, 512], F32)
    pstb = nc.alloc_psum_tensor("pstb", [P, 512], F32)
    psw = nc.alloc_psum_tensor("psw", [P, 512], F32)

    # ---- semaphores ----
    mmf = nc.alloc_semaphore("mmf")
    mmb = nc.alloc_semaphore("mmb")
    rawf = nc.alloc_semaphore("rawf")
    rawb = nc.alloc_semaphore("rawb")
    tpf = nc.alloc_semaphore("tpf")
    tpb = nc.alloc_semaphore("tpb")
    snf = nc.alloc_semaphore("snf")
    snb = nc.alloc_semaphore("snb")
    wdf = nc.alloc_semaphore("wdf")
    wdb = nc.alloc_semaphore("wdb")
    rsf = [nc.alloc_semaphore(f"rsf{i}") for i in range(2)]
    rsb = [nc.alloc_semaphore(f"rsb{i}") for i in range(2)]
    lsem = nc.alloc_semaphore("lsem")
    psem = nc.alloc_semaphore("psem")
    hsf = [nc.alloc_semaphore(f"hsf{i}") for i in range(2)]
    hsb = [nc.alloc_semaphore(f"hsb{i}") for i in range(2)]
    dma0 = nc.alloc_semaphore("dma0")

    pe, dve, act, gp, sp = nc.tensor, nc.vector, nc.scalar, nc.gpsimd, nc.sync

    # ---- prologue ----
    gp.memset(recf_sb[0:1, :], 1.0)
    gp.memset(recb_sb[0:1, :], 1.0)
    gp.memset(grawc[:, :], 0.0)
    gp.memset(sendc[:, :], 0.0)
    sp.dma_start(out=movf_sb[:, :], in_=movf[:, :]).then_inc(dma0, 16)
    sp.dma_start(out=movb_sb[:, :], in_=movb[:, :]).then_inc(dma0, 16)
    sp.dma_start(out=wf_sb[:, :], in_=wfi[:, :]).then_inc(dma0, 16)
    sp.dma_start(out=wb_sb[:, :], in_=wbi[:, :]).then_inc(dma0, 16)
    bs = blk_steps * MT
    sp.dma_start(out=hqf_sb[:, 0:bs], in_=hqf[:, 0:bs]).then_inc(hsf[0], 16)
    sp.dma_start(out=hqb_sb[:, 0:bs], in_=hqb[:, 0:bs]).then_inc(hsb[0], 16)
    if seq_blocks > 1:
        sp.dma_start(out=hqf_sb[:, bs : 2 * bs], in_=hqf[:, bs : 2 * bs]).then_inc(hsf[1], 16)
        sp.dma_start(out=hqb_sb[:, bs : 2 * bs], in_=hqb[:, bs : 2 * bs]).then_inc(hsb[1], 16)
    pe.wait_ge(dma0, 64)
    if not simulate:
        nc.all_core_barrier()

    # ---- registers ----
    def reg(engine, name, val=0):
        r = engine.alloc_register(name)
        engine.reg_mov(r, val)
        return r

    pwd_f = reg(pe, "pwd_f")
    pwd_b = reg(pe, "pwd_b")
    praw_f = reg(pe, "praw_f")
    praw_b = reg(pe, "praw_b")
    prec_f = reg(pe, "prec_f")
    prec_b = reg(pe, "prec_b")
    vtp_f = reg(dve, "vtp_f")
    vtp_b = reg(dve, "vtp_b")
    vmm_f = reg(dve, "vmm_f")
    vmm_b = reg(dve, "vmm_b")
    vls = reg(dve, "vls")
    vrs_f = [reg(dve, f"vrs_f{i}") for i in range(2)]
    vrs_b = [reg(dve, f"vrs_b{i}") for i in range(2)]
    vrec_f = reg(act, "vrec_f", 1)
    vrec_b = reg(act, "vrec_b", 1)
    vhq_f = reg(dve, "vhq_f")
    vhq_b = reg(dve, "vhq_b")
    vhs_f = reg(dve, "vhs_f")
    vhs_b = reg(dve, "vhs_b")
    amm_f = reg(act, "amm_f")
    amm_b = reg(act, "amm_b")
    gsn_f = reg(gp, "gsn_f")
    gsn_b = reg(gp, "gsn_b")
    gpd = [reg(gp, f"gpd{i}") for i in range(2)]
    s_src = reg(sp, "s_src")
    s_tmp = reg(sp, "s_tmp")
    s_cond = reg(sp, "s_cond")
    # broadcast dest offsets: graw col 2*pid within the parity half
    g_off2 = [gp.alloc_register(f"g_off2_{i}") for i in range(4)]
    gp.reg_alu(g_off2[0], gp.partition_id(), 4, op=mybir.AluOpType.mult)
    gp.reg_add(g_off2[1], g_off2[0], 2)    # (par0, b)
    gp.reg_add(g_off2[2], g_off2[0], 32)   # (par1, f)
    gp.reg_add(g_off2[3], g_off2[0], 34)   # (par1, b)

    hq_pitch = 2 * blk_steps * MT
    rec_pitch = seqh + 2

    def chain(par, co, qn, w_sb, mov_sb, ps, pst, row, rec_sb, hq_sb,
              s_mm, s_raw, s_tp, s_sn, s_wd, s_rs, p_wd, p_raw, p_rec,
              a_mm, v_tp, v_mm, v_rs, v_rec, v_hq, g_sn):
        """Emit one step of one chain; pieces are interleaved by caller."""

        def pe_mm():
            pe.wait_ge(s_wd, p_wd)
            pe.reg_add(p_wd, p_wd, 1)
            for j in range(NMM):
                pe.matmul(
                    ps[par][0:1, 0:MCOLS],
                    bass.AP(w_sb, j, [[32, P], [16, 2], [1, 1]]),
                    bass.AP(mov_sb, j * MO, [[NMM * MO, P], [MPAD, 2], [1, MCOLS]]),
                    start=(j == 0),
                    stop=(j == NMM - 1),
                    perf_mode=mybir.MatmulPerfMode.DoubleRow,
                ).then_maybe_inc((s_mm, 1) if j == NMM - 1 else None)

        def pe_tp():
            if not trans:
                return
            pe.reg_add(p_raw, p_raw, 1)
            pe.wait_ge(s_raw, p_raw)
            rec_ap = bass.AP(rec_sb, p_rec, [[rec_pitch, 1], [1, 1]])
            pe.matmul(
                pst[0:P, 0:1],
                bass.AP(row, par * 256, [[512, 1], [1, P]]),
                rec_ap, start=True, stop=True,
            )
            pe.matmul(
                pst[0:P, 1:2],
                bass.AP(row, par * 256 + P, [[512, 1], [1, P]]),
                rec_ap, start=True, stop=True,
            ).then_inc(s_tp, 1)
            pe.reg_add(p_rec, p_rec, 1)

        def act_row():
            if not trans:
                return
            act.reg_add(a_mm, a_mm, 1)
            act.wait_ge(s_mm, a_mm)
            act.activation(
                bass.AP(row, par * 256, [[512, 1], [1, 256]]),
                ps[par][0:1, 0:256],
                mybir.ActivationFunctionType.Copy,
            ).then_inc(s_raw, 1)

        def dve_row():
            return

        def act_recip():
            if not trans:
                return
            # stale-slot record: rec[t+1] = 1/S_t (FIFO after rowcopy => mm
            # done). Raw InstActivation: bass blocks ACT-Reciprocal for
            # accuracy, but recorded==applied here so any approximation
            # cancels exactly in the host correction.
            imm = lambda v: mybir.ImmediateValue(dtype=mybir.dt.float32, value=v)
            act.add_instruction(
                mybir.InstActivation(
                    name=nc.get_next_instruction_name(),
                    func=mybir.ActivationFunctionType.Reciprocal,
                    ins=[
                        act.lower_ap(
                            # sim: zero psum would trip the interp's recip
                            # range assert; timing-equivalent SBUF read
                            rec_sb[0:1, 0:1] if simulate
                            else ps[par][0:1, SLICE : SLICE + 1]
                        ),
                        imm(0.0), imm(1.0), imm(0.0),
                    ],
                    outs=[
                        act.lower_ap(
                            bass.AP(rec_sb, v_rec, [[rec_pitch, 1], [1, 1]])
                        )
                    ],
                )
            )
            act.reg_add(v_rec, v_rec, 1)

        def dve_send():
            if not trans:
                return
            dve.reg_add(v_tp, v_tp, 1)
            dve.wait_ge(s_tp, v_tp)
            base = 4 * par + co
            dve.tensor_copy(sendc[:, base : base + 2], pst[0:P, 0:2]).then_inc(s_sn, 1)
            if not comm:
                # timing-only local delivery (slot 0; wrong data)
                dve.tensor_copy(
                    bass.AP(grawc, par * 32 + co, [[64, P], [1, 2]]), pst[0:P, 0:2]
                )

        def dve_w():
            if comm and rswait:
                dve.reg_add(v_rs[par], v_rs[par], 2 if simulate else 16)
                dve.wait_ge(s_rs[par], v_rs[par])
            elif trans and not comm and co == 0:
                dve.drain()
            dve.tensor_tensor(
                bass.AP(w_sb, 0, [[32, P], [1, 8], [16, 2]]),
                bass.AP(grawc, par * 32 + co, [[64, P], [4, 8], [1, 2]]),
                bass.AP(hq_sb, v_hq, [[hq_pitch, P], [1, MT]]),
                op=mybir.AluOpType.mult,
            ).then_inc(s_wd, 1)
            dve.reg_add(v_hq, v_hq, MT)

        def gp_prep():
            if not comm:
                return
            gp.remote_dma_broadcast(
                out_ap=bass.AP(grawc, g_off2[2 * par + (co // 2)], [[64, P], [1, 2]]),
                in_ap=sendc[:, 4 * par + co : 4 * par + co + 2],
                remote_sem=s_rs[par],
                local_sem=lsem,
                queue_num=qn,
                rdests=[(0, k) for k in range(NCORES)],
            ).then_inc(psem, 1)

        def gp_trig():
            if not comm:
                return
            gp.reg_add(g_sn, g_sn, 1)
            gp.wait_ge(s_sn, g_sn)
            gp.trigger_dma(count=1, queue_num=qn)

        return pe_mm, pe_tp, act_row, dve_row, act_recip, dve_send, dve_w, gp_prep, gp_trig

    def emit_round(par):
        f = chain(par, 0, 0, wf_sb, movf_sb, psf, pstf, rowf, recf_sb,
                  hqf_sb, mmf, rawf, tpf, snf, wdf, rsf, pwd_f, praw_f,
                  prec_f, amm_f, vtp_f, vmm_f, vrs_f, vrec_f, vhq_f, gsn_f)
        b = chain(par, 2, 0, wb_sb, movb_sb, psb, pstb, rowb, recb_sb,
                  hqb_sb, mmb, rawb, tpb, snb, wdb, rsb, pwd_b, praw_b,
                  prec_b, amm_b, vtp_b, vmm_b, vrs_b, vrec_b, vhq_b, gsn_b)
        f_mm, f_tp, f_row, f_drow, f_recip, f_send, f_w, f_prep, f_trig = f
        b_mm, b_tp, b_row, b_drow, b_recip, b_send, b_w, b_prep, b_trig = b

        # PE: both matvecs back-to-back, then both transposes
        f_mm()
        b_mm()
        f_tp()
        b_tp()
        # HAM warm-fill: self-paced dummy matmuls run immediately after the
        # transposes, stretching PE activity across the comm gap so the clock
        # gate never observes a full idle window (keeps PE at 2.4 GHz). They
        # write a dedicated scratch psum bank and have no consumers.
        if comm and not simulate:
            for d in range(6):
                pe.matmul(
                    psw[0:1, 0:448],
                    bass.AP(rowf, 0, [[512, 1], [1, 1]]),
                    bass.AP(rowf, 0, [[512, 1], [1, 448]]),
                    start=True, stop=True,
                )

        # ACT: rowcopies first (critical path), recips trail; DVE mirrors
        # the second halves in parallel
        f_row()
        b_row()
        f_recip()
        b_recip()
        # DVE: rounds 2k/2k+1 reuse send buffers of rounds 2k-2/2k-1; one
        # pair-top wait for all prior-pair sends drained covers both
        if comm and par == 0:
            dve.wait_ge(lsem, vls)
            dve.reg_add(vls, vls, 64)
        f_drow()
        b_drow()
        f_send()
        b_send()
        f_w()
        b_w()
        # GP: preps at round top would be better, but keep trigger order simple
        f_prep()
        b_prep()
        if comm:
            gp.reg_add(gpd[0], gpd[0], 2)
            gp.wait_ge(psem, gpd[0])
        f_trig()
        b_trig()

    # ---- main loop ----
    with nc.Fori(0, seq_blocks // 2) as g:
        for p01 in range(2):
            # gate this block's h DMAs; set read offsets
            for v_hs, v_hq, hs in ((vhs_f, vhq_f, hsf), (vhs_b, vhq_b, hsb)):
                dve.reg_alu(v_hs, g, 16, op=mybir.AluOpType.mult)
                dve.reg_add(v_hs, v_hs, 16)
                dve.wait_ge(hs[p01], v_hs)
                dve.reg_mov(v_hq, p01 * blk_steps * MT)

            # prefetch block 2g+2+p01 once block 2g+p01 is consumed
            if seq_blocks > 2:
                for hq_par, hq_dram, s_wd_, hs in (
                    (hqf_sb, hqf, wdf, hsf), (hqb_sb, hqb, wdb, hsb)
                ):
                    sp.reg_alu(s_tmp, g, 2 * blk_steps, op=mybir.AluOpType.mult)
                    sp.reg_add(s_tmp, s_tmp, (1 + p01) * blk_steps)
                    sp.reg_mov(s_cond, 0)
                    sp.reg_add(s_cond, g, 0)
                    with sp.If_lt(s_cond, seq_blocks // 2 - 1):
                        sp.wait_ge(s_wd_, s_tmp)
                        sp.reg_add(s_src, g, 0)
                        sp.reg_alu(s_src, s_src, 2 * blk_steps * MT, op=mybir.AluOpType.mult)
                        sp.reg_add(s_src, s_src, (2 + p01) * blk_steps * MT)
                        sp.dma_start(
                            out=bass.AP(hq_par, p01 * blk_steps * MT,
                                        [[2 * blk_steps * MT, P], [1, blk_steps * MT]]),
                            in_=bass.AP(hq_dram, s_src, [[seqh * MT, P], [1, blk_steps * MT]]),
                        ).then_inc(hs[p01], 16)

            with nc.Fori(0, blk_steps // 2):
                emit_round(0)
                emit_round(1)

    # ---- epilogue ----
    sp.wait_ge(wdf, seqh)
    sp.wait_ge(wdb, seqh)
    sp.dma_start(out=wfo[:, :], in_=wf_sb[:, :]).then_inc(dma0, 16)
    sp.dma_start(out=wbo[:, :], in_=wb_sb[:, :]).then_inc(dma0, 16)
    sp.dma_start(out=recf_o[:, :], in_=recf_sb[:, :]).then_inc(dma0, 16)
    sp.dma_start(out=recb_o[:, :], in_=recb_sb[:, :]).then_inc(dma0, 16)
    sp.wait_ge(dma0, 128)
    if comm:
        gp.wait_ge(lsem, 32 * seqh)
    if not simulate:
        nc.all_core_barrier()
    nc.finalize()
    return nc


def _col_layout(vec_by_chunk):
    """[16, 128] chunk-major values -> [128, 32] fp8 DR pair-stride-16 layout:
    chunk c at column c//2 + 16*(c%2)."""
    out = np.zeros((P, 32), dtype=np.float32)
    for c in range(MT):
        out[:, (c // 2) + 16 * (c % 2)] = vec_by_chunk[c]
    return out


def _mov_tiles(A, r):
    """Moving tiles for core r from matvec matrix A [2048 next, 2048 prev]
    (already ALPHA-scaled): [128, NMM*MO] fp8.
    mov[q, j*MO + i*MPAD + n] = A[256 r + n, 128 (2j+i) + q]; col 256 = colsum."""
    colsum = A.sum(axis=0)                                   # [2048]
    Asl = A[SLICE * r : SLICE * (r + 1), :]                  # [256, 2048]
    out = np.zeros((P, NMM * MO), dtype=np.float32)
    for j in range(NMM):
        for i in range(2):
            c = 2 * j + i
            blk = Asl[:, P * c : P * (c + 1)]                # [256 n, 128 q]
            base = j * MO + i * MPAD
            out[:, base : base + SLICE] = blk.T              # [q, n]
            out[:, base + SLICE] = colsum[P * c : P * (c + 1)]
    return out.astype(NPFP8)


def prep_inputs(h, transitions, seqh):
    h32 = np.asarray(h, dtype=np.float32)
    T32 = np.asarray(transitions, dtype=np.float32)
    expT = np.exp(T32.astype(np.float64))
    Af = (ALPHA * expT).astype(np.float32)
    Ab = (ALPHA * expT.T).astype(np.float32)

    eh = np.exp(h32.astype(np.float64)) * BETA               # [8192, 2048]

    # fwd emissions: rows 0..seqh-1; bwd: rows 8190..4096 then ones
    def hq_layout(rows):
        # rows: [seqh, 2048] -> [128, seqh*16]: hq[q, t*16+c] = rows[t, 128c+q]
        r = rows.reshape(seqh, MT, P).transpose(2, 0, 1)     # [q, t, c]
        return np.ascontiguousarray(r.reshape(P, seqh * MT)).astype(NPBF16)

    hqf = hq_layout(eh[0:seqh].astype(np.float32))
    bwd_rows = np.empty((seqh, K), dtype=np.float32)
    bwd_rows[: seqh - 1] = eh[2 * seqh - 2 : seqh - 1 : -1]  # rows 8190..4096
    bwd_rows[seqh - 1] = 1.0
    hqb = hq_layout(bwd_rows)

    # inits
    wf0 = np.zeros((MT, P), dtype=np.float32)
    wf0[0, START_IDX] = 1.0
    wfi = _col_layout(wf0).astype(NPFP8)
    y0 = eh[2 * seqh - 1] * expT[END_IDX, :]                         # [2048]
    c0 = float(y0.sum() / 26.0)                  # normalize toward equilibrium
    y0 = (y0 / c0).astype(np.float32)
    wbi = _col_layout(y0.reshape(MT, P)).astype(NPFP8)

    in_maps = []
    for r in range(NCORES):
        in_maps.append({
            "movf": _mov_tiles(Af, r),
            "movb": _mov_tiles(Ab, r),
            "hqf": hqf,
            "hqb": hqb,
            "wfi": wfi,
            "wbi": wbi,
        })
    return in_maps, c0


def finalize(results, seqh, c0):
    res = results[0]

    def unvec(w8):
        w = w8.astype(np.float64)                            # [128, 32]
        v = np.empty(K, dtype=np.float64)
        for c in range(MT):
            v[P * c : P * (c + 1)] = w[:, (c // 2) + 16 * (c % 2)]
        return v

    wf = unvec(res["wfo"])
    yb = unvec(res["wbo"])
    dot = float(np.dot(wf, yb))
    recf = res["recf"].reshape(-1).astype(np.float64)[0:seqh]
    recb = res["recb"].reshape(-1).astype(np.float64)[0:seqh]
    ans = (
        np.log(dot)
        + np.log(c0)
        - np.sum(np.log(recf))
        - np.sum(np.log(recb))
        - 2.0 * seqh * np.log(ALPHA)
        - 2.0 * seqh * np.log(BETA)
    )
    return np.float32(ans)


def kernel(h: np.ndarray, transitions: np.ndarray) -> np.ndarray:
    from concourse.bass_utils import run_bass_kernel_spmd

    seq_blocks, blk_steps = 16, SEQH // 16
    nc = build_bass(seq_blocks, blk_steps)
    in_maps, c0 = prep_inputs(np.asarray(h), np.asarray(transitions), SEQH)
    res = run_bass_kernel_spmd(nc, in_maps, core_ids=list(range(NCORES)))
    return finalize(res.results, SEQH, c0)


if __name__ == "__main__":
    rng = np.random.default_rng(0)
    out = kernel(
        rng.standard_normal((SEQ, K)).astype(np.float32),
        rng.standard_normal((K, K)).astype(np.float32),
    )
    print("kernel:", out)
